# revision 14
# baseline (speedup 1.0000x reference)
import numpy as np
import ml_dtypes

M = 16384
N = 16384
NCORES = 8
C = 256
U = 2
GROUP = 6
CP = 128
NWARM = 32

_cache = {}


def _split2(v):
    hi = v.astype(ml_dtypes.bfloat16)
    lo = (v - hi.astype(np.float64)).astype(ml_dtypes.bfloat16)
    return hi, lo


def _split3(v):
    hi = v.astype(ml_dtypes.bfloat16)
    r = v - hi.astype(np.float64)
    mid = r.astype(ml_dtypes.bfloat16)
    lo = (r - mid.astype(np.float64)).astype(ml_dtypes.bfloat16)
    return hi, mid, lo


def _kd_chunks(X, leaf):
    chunks = []

    def rec(ids):
        if len(ids) <= leaf:
            chunks.append(ids)
            return
        ax = int(np.argmax(X[ids].max(0) - X[ids].min(0)))
        order = ids[np.argsort(X[ids, ax], kind="stable")]
        h = len(order) // 2
        rec(order[:h])
        rec(order[h:])

    rec(np.arange(len(X)))
    return chunks


def _prep(X_test, X_train, alpha, log_lengthscale, log_outputscale):
    ell = np.exp(np.float32(log_lengthscale))
    ell2 = np.float64(np.float32(ell) ** 2)
    sf = np.exp(np.float32(log_outputscale))
    sf2 = np.float64(np.float32(sf) ** 2)
    bias = float(np.float32(np.log(sf2)))

    xt = X_train.astype(np.float64)
    xs = X_test.astype(np.float64)
    al = alpha.astype(np.float64)

    x0h, x0l = _split2(xt[:, 0])
    x1h, x1l = _split2(xt[:, 1])
    pj = -(xt[:, 0] ** 2 + xt[:, 1] ** 2) / (2.0 * ell2)
    pjh, pjm, pjl = _split3(pj)
    ones = np.ones(N, dtype=ml_dtypes.bfloat16)
    A14 = np.stack(
        [ones, ones, ones, x0h, x0h, x0l, x0l, x1h, x1h, x1l, x1l, pjh, pjm, pjl]
    )

    T0 = -(xs[:, 0] ** 2 + xs[:, 1] ** 2) / (2.0 * ell2)
    T0h, T0m, T0l = _split3(T0)
    u0 = xs[:, 0] / ell2
    u0h, u0l = _split2(u0)
    u1 = xs[:, 1] / ell2
    u1h, u1l = _split2(u1)
    onesM = np.ones(M, dtype=ml_dtypes.bfloat16)
    B14 = np.stack(
        [T0h, T0m, T0l, u0h, u0l, u0h, u0l, u1h, u1l, u1h, u1l, onesM, onesM, onesM]
    )

    arh, arl = _split2(al[:, 0])
    aih, ail = _split2(al[:, 1])
    AL4 = np.stack([arh, arl, aih, ail], axis=1).astype(ml_dtypes.bfloat16)

    rcut = float(3.203 * ell)
    chunks = _kd_chunks(np.asarray(X_test, dtype=np.float64), C)

    cells = []
    for ids in chunks:
        lo = xs[ids].min(0) - rcut
        hi = xs[ids].max(0) + rcut
        box = np.where(
            (xt[:, 0] >= lo[0])
            & (xt[:, 0] <= hi[0])
            & (xt[:, 1] >= lo[1])
            & (xt[:, 1] <= hi[1])
        )[0]
        if len(box):
            d2 = (
                (xs[ids][:, None, :] - xt[box][None, :, :]) ** 2
            ).sum(-1).min(0)
            w = box[d2 <= rcut * rcut]
        else:
            w = box
        t = max(1, (len(w) + 127) // 128)
        for tl in range(0, t, U):
            cells.append((ids, w, tl))

    ncell = (len(cells) + NCORES - 1) // NCORES
    Scells = ncell * NCORES
    S = ncell * U

    core_maps = []
    core_meta = []
    for c in range(NCORES):
        my = cells[c * ncell : (c + 1) * ncell]
        gidx = np.zeros(S * 128, dtype=np.int64)
        alz = np.ones(S * 128, dtype=bool)
        Bcols = np.zeros(ncell * C, dtype=np.int64)
        meta = []
        for i in range(ncell):
            if i < len(my):
                ids, w, tl = my[i]
                npts_all = len(w)
                lo_pt = tl * 128
                seg = w[lo_pt : lo_pt + U * 128]
                filler = int(w[0]) if npts_all else 0
                blk = np.full(U * 128, filler, dtype=np.int64)
                blk[: len(seg)] = seg
                sl = slice(i * U * 128, (i + 1) * U * 128)
                gidx[sl] = blk
                alz[sl.start : sl.start + len(seg)] = False
                Bcols[i * C : (i + 1) * C] = ids
                meta.append(ids)
            else:
                meta.append(None)
        A_core = np.ascontiguousarray(A14[:, gidx])
        ALg = AL4[gidx].copy()
        ALg[alz] = 0
        AL_core = np.ascontiguousarray(
            ALg.reshape(S, 128, 4).transpose(1, 0, 2).reshape(128, S * 4)
        )
        B_core = np.ascontiguousarray(B14[:, Bcols])
        core_maps.append({"A": A_core, "B": B_core, "AL": AL_core})
        core_meta.append(meta)

    return {
        "bias": bias,
        "S": S,
        "ncell": ncell,
        "core_maps": core_maps,
        "core_meta": core_meta,
    }


def _build_program(bias, S):
    import concourse.mybir as mybir
    import concourse.tile as tile
    from concourse import bacc

    fp32 = mybir.dt.float32
    bf16 = mybir.dt.bfloat16

    ncell = S // U
    ng4 = (ncell + 3) // 4
    groups = [
        list(range(g * GROUP, min(S, (g + 1) * GROUP)))
        for g in range((S + GROUP - 1) // GROUP)
    ]

    nc = bacc.Bacc(None, target_bir_lowering=False)
    A_d = nc.declare_dram_parameter("A", [14, S * 128], bf16, isOutput=False)
    B_d = nc.declare_dram_parameter("B", [14, ncell * C], bf16, isOutput=False)
    AL_d = nc.declare_dram_parameter("AL", [128, S * 4], bf16, isOutput=False)
    OUT_d = nc.declare_dram_parameter("out", [36, ng4 * 512], fp32, isOutput=True)

    with tile.TileContext(nc) as tc:
        with (
            tc.tile_pool(name="singles", bufs=1) as singles,
            tc.tile_pool(name="kpool", bufs=3) as kpool,
            tc.tile_pool(name="stpool", bufs=2) as stpool,
            tc.tile_pool(name="pse", bufs=2, space="PSUM") as pse,
            tc.tile_pool(name="psacc", bufs=2, space="PSUM") as psacc,
        ):
            sb_B = singles.tile([CP, ncell * C], bf16)
            sb_A = singles.tile([CP, S * 128], bf16)
            sb_AL = singles.tile([128, S * 4], bf16)
            wtile = singles.tile([128, 128], bf16)

            nc.vector.memset(wtile, 0)
            FR = [0.0, 0.125, 0.25, 0.5, 1.0]
            asl = [int(f * S * 128) for f in FR]
            bsl = [int(f * ncell * C) for f in FR]
            for i in range(len(FR) - 1):
                nc.vector.memset(sb_A[:, asl[i] : asl[i + 1]], 0)
                nc.vector.memset(sb_B[:, bsl[i] : bsl[i + 1]], 0)

            for i in range(len(FR) - 1):
                eng = nc.sync if i % 2 == 0 else nc.gpsimd
                eng.dma_start(
                    out=sb_B[:14, bsl[i] : bsl[i + 1]], in_=B_d[:, bsl[i] : bsl[i + 1]]
                )
                eng.dma_start(
                    out=sb_A[:14, asl[i] : asl[i + 1]], in_=A_d[:, asl[i] : asl[i + 1]]
                )
                if i == 0:
                    nc.gpsimd.dma_start(out=sb_AL, in_=AL_d[:])

            ew = pse.tile([128, GROUP * C], fp32, name="e")
            for i in range(NWARM):
                nc.tensor.matmul(
                    ew[:, :128], lhsT=wtile, rhs=wtile, start=True, stop=True
                )

            acc = None
            st = None
            for gs in groups:
                W = len(gs) * C
                e = pse.tile([128, GROUP * C], fp32, name="e")
                for i, s in enumerate(gs):
                    cell = s // U
                    nc.tensor.matmul(
                        e[:, i * C : (i + 1) * C],
                        lhsT=sb_A[:, s * 128 : (s + 1) * 128],
                        rhs=sb_B[:, cell * C : (cell + 1) * C],
                        start=True,
                        stop=True,
                    )
                k = kpool.tile([128, GROUP * C], bf16, name="k")
                nc.scalar.activation(
                    k[:, :W], e[:, :W], mybir.ActivationFunctionType.Exp,
                    bias=float(bias),
                )
                for i, s in enumerate(gs):
                    cell = s // U
                    g4, r = cell // 4, cell % 4
                    if acc is None:
                        acc = psacc.tile([36, 512], fp32, name="acc")
                    nc.tensor.matmul(
                        acc[32 * (r // 2) : 32 * (r // 2) + 4,
                            256 * (r % 2) : 256 * (r % 2) + 256],
                        lhsT=sb_AL[:, s * 4 : (s + 1) * 4],
                        rhs=k[:, i * C : (i + 1) * C],
                        start=(s % U == 0),
                        stop=(s % U == U - 1),
                    )
                    if s % U == U - 1 and (r == 3 or cell == ncell - 1):
                        st = stpool.tile([36, 512], fp32, name="st")
                        nc.vector.tensor_copy(st, acc)
                        nc.sync.dma_start(
                            out=OUT_d[:, g4 * 512 : (g4 + 1) * 512], in_=st
                        )
                        acc = None
    nc.compile()
    return nc


def _unpack(results, prep):
    out = np.zeros((M, 2), dtype=np.float32)
    for c in range(NCORES):
        o = results[c]["out"]
        for i, ids in enumerate(prep["core_meta"][c]):
            if ids is None:
                continue
            g4, r = i // 4, i % 4
            blk = o[
                32 * (r // 2) : 32 * (r // 2) + 4,
                g4 * 512 + 256 * (r % 2) : g4 * 512 + 256 * (r % 2) + C,
            ]
            out[ids, 0] += blk[0] + blk[1]
            out[ids, 1] += blk[2] + blk[3]
    return out


def kernel(X_test, X_train, alpha, log_lengthscale, log_outputscale):
    from concourse.bass_utils import run_bass_kernel_spmd

    prep = _prep(X_test, X_train, alpha, log_lengthscale, log_outputscale)

    key = (prep["S"], prep["bias"])
    if key not in _cache:
        _cache[key] = _build_program(prep["bias"], prep["S"])
    nc = _cache[key]

    core_ids = list(range(NCORES))
    res = run_bass_kernel_spmd(nc, prep["core_maps"], core_ids)
    return _unpack(res.results, prep)


# revision 16
# speedup vs baseline: 1.1333x; 1.1333x over previous
import numpy as np
import ml_dtypes

M = 16384
N = 16384
NCORES = 8
C = 256
U = 2
GROUP = 6
KP = 32
NWARM = 32

_cache = {}


def _split2(v):
    hi = v.astype(ml_dtypes.bfloat16)
    lo = (v - hi.astype(np.float64)).astype(ml_dtypes.bfloat16)
    return hi, lo


def _split3(v):
    hi = v.astype(ml_dtypes.bfloat16)
    r = v - hi.astype(np.float64)
    mid = r.astype(ml_dtypes.bfloat16)
    lo = (r - mid.astype(np.float64)).astype(ml_dtypes.bfloat16)
    return hi, mid, lo


def _kd_chunks(X, leaf):
    chunks = []

    def rec(ids):
        if len(ids) <= leaf:
            chunks.append(ids)
            return
        ax = int(np.argmax(X[ids].max(0) - X[ids].min(0)))
        order = ids[np.argsort(X[ids, ax], kind="stable")]
        h = len(order) // 2
        rec(order[:h])
        rec(order[h:])

    rec(np.arange(len(X)))
    return chunks


def _prep(X_test, X_train, alpha, log_lengthscale, log_outputscale):
    ell = np.exp(np.float32(log_lengthscale))
    ell2 = np.float64(np.float32(ell) ** 2)
    sf = np.exp(np.float32(log_outputscale))
    sf2 = np.float64(np.float32(sf) ** 2)
    bias = float(np.float32(np.log(sf2)))

    xt = X_train.astype(np.float64)
    xs = X_test.astype(np.float64)
    al = alpha.astype(np.float64)

    x0h, x0l = _split2(xt[:, 0])
    x1h, x1l = _split2(xt[:, 1])
    pj = -(xt[:, 0] ** 2 + xt[:, 1] ** 2) / (2.0 * ell2)
    pjh, pjm, pjl = _split3(pj)
    ones = np.ones(N, dtype=ml_dtypes.bfloat16)
    A14 = np.stack(
        [ones, ones, ones, x0h, x0h, x0l, x0l, x1h, x1h, x1l, x1l, pjh, pjm, pjl]
    )

    T0 = -(xs[:, 0] ** 2 + xs[:, 1] ** 2) / (2.0 * ell2)
    T0h, T0m, T0l = _split3(T0)
    u0 = xs[:, 0] / ell2
    u0h, u0l = _split2(u0)
    u1 = xs[:, 1] / ell2
    u1h, u1l = _split2(u1)
    onesM = np.ones(M, dtype=ml_dtypes.bfloat16)
    B14 = np.stack(
        [T0h, T0m, T0l, u0h, u0l, u0h, u0l, u1h, u1l, u1h, u1l, onesM, onesM, onesM]
    )

    arh, arl = _split2(al[:, 0])
    aih, ail = _split2(al[:, 1])
    AL4 = np.stack([arh, arl, aih, ail], axis=1).astype(ml_dtypes.bfloat16)

    rcut = float(3.203 * ell)
    chunks = _kd_chunks(np.asarray(X_test, dtype=np.float64), C)

    cells = []
    for ids in chunks:
        lo = xs[ids].min(0) - rcut
        hi = xs[ids].max(0) + rcut
        box = np.where(
            (xt[:, 0] >= lo[0])
            & (xt[:, 0] <= hi[0])
            & (xt[:, 1] >= lo[1])
            & (xt[:, 1] <= hi[1])
        )[0]
        if len(box):
            d2 = (
                (xs[ids][:, None, :] - xt[box][None, :, :]) ** 2
            ).sum(-1).min(0)
            w = box[d2 <= rcut * rcut]
        else:
            w = box
        t = max(1, (len(w) + 127) // 128)
        for tl in range(0, t, U):
            cells.append((ids, w, tl))

    ncell = (len(cells) + NCORES - 1) // NCORES
    S = ncell * U

    core_maps = []
    core_meta = []
    for c in range(NCORES):
        my = cells[c * ncell : (c + 1) * ncell]
        gidx = np.zeros(S * 128, dtype=np.int64)
        alz = np.ones(S * 128, dtype=bool)
        Bcols = np.zeros(ncell * C, dtype=np.int64)
        meta = []
        for i in range(ncell):
            if i < len(my):
                ids, w, tl = my[i]
                npts_all = len(w)
                seg = w[tl * 128 : tl * 128 + U * 128]
                filler = int(w[0]) if npts_all else 0
                blk = np.full(U * 128, filler, dtype=np.int64)
                blk[: len(seg)] = seg
                sl = slice(i * U * 128, (i + 1) * U * 128)
                gidx[sl] = blk
                alz[sl.start : sl.start + len(seg)] = False
                Bcols[i * C : (i + 1) * C] = ids
                meta.append(ids)
            else:
                meta.append(None)
        A_core = np.zeros((KP, S * 128), dtype=ml_dtypes.bfloat16)
        A_core[:14] = A14[:, gidx]
        ALg = AL4[gidx].copy()
        ALg[alz] = 0
        AL_core = np.ascontiguousarray(
            ALg.reshape(S, 128, 4).transpose(1, 0, 2).reshape(128, S * 4)
        )
        B_core = np.zeros((KP, ncell * C), dtype=ml_dtypes.bfloat16)
        B_core[:14] = B14[:, Bcols]
        core_maps.append({"A": A_core, "B": B_core, "AL": AL_core})
        core_meta.append(meta)

    return {
        "bias": bias,
        "S": S,
        "ncell": ncell,
        "core_maps": core_maps,
        "core_meta": core_meta,
    }


def _build_program(bias, S):
    import concourse.mybir as mybir
    import concourse.tile as tile
    from concourse import bacc

    fp32 = mybir.dt.float32
    bf16 = mybir.dt.bfloat16

    ncell = S // U
    ng4 = (ncell + 3) // 4
    groups = [
        list(range(g * GROUP, min(S, (g + 1) * GROUP)))
        for g in range((S + GROUP - 1) // GROUP)
    ]

    nc = bacc.Bacc(None, target_bir_lowering=False)
    A_d = nc.declare_dram_parameter("A", [KP, S * 128], bf16, isOutput=False)
    B_d = nc.declare_dram_parameter("B", [KP, ncell * C], bf16, isOutput=False)
    AL_d = nc.declare_dram_parameter("AL", [128, S * 4], bf16, isOutput=False)
    OUT_d = nc.declare_dram_parameter("out", [36, ng4 * 512], fp32, isOutput=True)

    with tile.TileContext(nc) as tc:
        with (
            tc.tile_pool(name="singles", bufs=1) as singles,
            tc.tile_pool(name="kpool", bufs=3) as kpool,
            tc.tile_pool(name="stpool", bufs=2) as stpool,
            tc.tile_pool(name="pse", bufs=2, space="PSUM") as pse,
            tc.tile_pool(name="psacc", bufs=2, space="PSUM") as psacc,
        ):
            sb_B = singles.tile([KP, ncell * C], bf16)
            sb_A = singles.tile([KP, S * 128], bf16)
            sb_AL = singles.tile([128, S * 4], bf16)
            wtile = singles.tile([128, 128], bf16)

            nc.vector.memset(wtile, 0)

            FR = [0.0, 0.125, 0.25, 0.5, 1.0]
            asl = [int(f * S * 128) for f in FR]
            bsl = [int(f * ncell * C) for f in FR]
            for i in range(len(FR) - 1):
                eng = nc.sync if i % 2 == 0 else nc.gpsimd
                eng.dma_start(
                    out=sb_B[:, bsl[i] : bsl[i + 1]], in_=B_d[:, bsl[i] : bsl[i + 1]]
                )
                eng.dma_start(
                    out=sb_A[:, asl[i] : asl[i + 1]], in_=A_d[:, asl[i] : asl[i + 1]]
                )
                if i == 0:
                    nc.gpsimd.dma_start(out=sb_AL, in_=AL_d[:])

            ew = pse.tile([128, GROUP * C], fp32, name="e")
            for i in range(NWARM):
                nc.tensor.matmul(
                    ew[:, :128], lhsT=wtile, rhs=wtile, start=True, stop=True
                )

            acc = None
            for gs in groups:
                W = len(gs) * C
                e = pse.tile([128, GROUP * C], fp32, name="e")
                for i, s in enumerate(gs):
                    cell = s // U
                    nc.tensor.matmul(
                        e[:, i * C : (i + 1) * C],
                        lhsT=sb_A[:, s * 128 : (s + 1) * 128],
                        rhs=sb_B[:, cell * C : (cell + 1) * C],
                        start=True,
                        stop=True,
                    )
                k = kpool.tile([128, GROUP * C], bf16, name="k")
                nc.scalar.activation(
                    k[:, :W], e[:, :W], mybir.ActivationFunctionType.Exp,
                    bias=float(bias),
                )
                for i, s in enumerate(gs):
                    cell = s // U
                    g4, r = cell // 4, cell % 4
                    if acc is None:
                        acc = psacc.tile([36, 512], fp32, name="acc")
                    nc.tensor.matmul(
                        acc[32 * (r // 2) : 32 * (r // 2) + 4,
                            256 * (r % 2) : 256 * (r % 2) + 256],
                        lhsT=sb_AL[:, s * 4 : (s + 1) * 4],
                        rhs=k[:, i * C : (i + 1) * C],
                        start=(s % U == 0),
                        stop=(s % U == U - 1),
                    )
                    if s % U == U - 1 and (r == 3 or cell == ncell - 1):
                        st = stpool.tile([36, 512], fp32, name="st")
                        nc.vector.tensor_copy(st, acc)
                        nc.sync.dma_start(
                            out=OUT_d[:, g4 * 512 : (g4 + 1) * 512], in_=st
                        )
                        acc = None
    nc.compile()
    return nc


def _unpack(results, prep):
    out = np.zeros((M, 2), dtype=np.float32)
    for c in range(NCORES):
        o = results[c]["out"]
        for i, ids in enumerate(prep["core_meta"][c]):
            if ids is None:
                continue
            g4, r = i // 4, i % 4
            blk = o[
                32 * (r // 2) : 32 * (r // 2) + 4,
                g4 * 512 + 256 * (r % 2) : g4 * 512 + 256 * (r % 2) + C,
            ]
            out[ids, 0] += blk[0] + blk[1]
            out[ids, 1] += blk[2] + blk[3]
    return out


def kernel(X_test, X_train, alpha, log_lengthscale, log_outputscale):
    from concourse.bass_utils import run_bass_kernel_spmd

    prep = _prep(X_test, X_train, alpha, log_lengthscale, log_outputscale)

    key = (prep["S"], prep["bias"])
    if key not in _cache:
        _cache[key] = _build_program(prep["bias"], prep["S"])
    nc = _cache[key]

    core_ids = list(range(NCORES))
    res = run_bass_kernel_spmd(nc, prep["core_maps"], core_ids)
    return _unpack(res.results, prep)


# revision 18
# speedup vs baseline: 1.1935x; 1.0531x over previous
import numpy as np
import ml_dtypes

M = 16384
N = 16384
NCORES = 8
C = 256
U = 2
GROUP = 6
KP = 128
NWARM = 20

_cache = {}


def _split2(v):
    hi = v.astype(ml_dtypes.bfloat16)
    lo = (v - hi.astype(np.float64)).astype(ml_dtypes.bfloat16)
    return hi, lo


def _split3(v):
    hi = v.astype(ml_dtypes.bfloat16)
    r = v - hi.astype(np.float64)
    mid = r.astype(ml_dtypes.bfloat16)
    lo = (r - mid.astype(np.float64)).astype(ml_dtypes.bfloat16)
    return hi, mid, lo


def _kd_chunks(X, leaf):
    chunks = []

    def rec(ids):
        if len(ids) <= leaf:
            chunks.append(ids)
            return
        ax = int(np.argmax(X[ids].max(0) - X[ids].min(0)))
        order = ids[np.argsort(X[ids, ax], kind="stable")]
        h = len(order) // 2
        rec(order[:h])
        rec(order[h:])

    rec(np.arange(len(X)))
    return chunks


def _prep(X_test, X_train, alpha, log_lengthscale, log_outputscale):
    ell = np.exp(np.float32(log_lengthscale))
    ell2 = np.float64(np.float32(ell) ** 2)
    sf = np.exp(np.float32(log_outputscale))
    sf2 = np.float64(np.float32(sf) ** 2)
    bias = float(np.float32(np.log(sf2)))

    xt = X_train.astype(np.float64)
    xs = X_test.astype(np.float64)
    al = alpha.astype(np.float64)

    x0h, x0l = _split2(xt[:, 0])
    x1h, x1l = _split2(xt[:, 1])
    pj = -(xt[:, 0] ** 2 + xt[:, 1] ** 2) / (2.0 * ell2)
    pjh, pjm, pjl = _split3(pj)
    ones = np.ones(N, dtype=ml_dtypes.bfloat16)
    A14 = np.stack(
        [ones, ones, ones, x0h, x0h, x0l, x0l, x1h, x1h, x1l, x1l, pjh, pjm, pjl]
    )

    T0 = -(xs[:, 0] ** 2 + xs[:, 1] ** 2) / (2.0 * ell2)
    T0h, T0m, T0l = _split3(T0)
    u0 = xs[:, 0] / ell2
    u0h, u0l = _split2(u0)
    u1 = xs[:, 1] / ell2
    u1h, u1l = _split2(u1)
    onesM = np.ones(M, dtype=ml_dtypes.bfloat16)
    B14 = np.stack(
        [T0h, T0m, T0l, u0h, u0l, u0h, u0l, u1h, u1l, u1h, u1l, onesM, onesM, onesM]
    )

    arh, arl = _split2(al[:, 0])
    aih, ail = _split2(al[:, 1])
    AL4 = np.stack([arh, arl, aih, ail], axis=1).astype(ml_dtypes.bfloat16)

    rcut = float(3.203 * ell)
    chunks = _kd_chunks(np.asarray(X_test, dtype=np.float64), C)

    cells = []
    for ids in chunks:
        lo = xs[ids].min(0) - rcut
        hi = xs[ids].max(0) + rcut
        box = np.where(
            (xt[:, 0] >= lo[0])
            & (xt[:, 0] <= hi[0])
            & (xt[:, 1] >= lo[1])
            & (xt[:, 1] <= hi[1])
        )[0]
        if len(box):
            d2 = (
                (xs[ids][:, None, :] - xt[box][None, :, :]) ** 2
            ).sum(-1).min(0)
            w = box[d2 <= rcut * rcut]
        else:
            w = box
        t = max(1, (len(w) + 127) // 128)
        for tl in range(0, t, U):
            cells.append((ids, w, tl))

    ncell = (len(cells) + NCORES - 1) // NCORES
    S = ncell * U

    core_maps = []
    core_meta = []
    for c in range(NCORES):
        my = cells[c * ncell : (c + 1) * ncell]
        gidx = np.zeros(S * 128, dtype=np.int64)
        alz = np.ones(S * 128, dtype=bool)
        Bcols = np.zeros(ncell * C, dtype=np.int64)
        meta = []
        for i in range(ncell):
            if i < len(my):
                ids, w, tl = my[i]
                npts_all = len(w)
                seg = w[tl * 128 : tl * 128 + U * 128]
                filler = int(w[0]) if npts_all else 0
                blk = np.full(U * 128, filler, dtype=np.int64)
                blk[: len(seg)] = seg
                sl = slice(i * U * 128, (i + 1) * U * 128)
                gidx[sl] = blk
                alz[sl.start : sl.start + len(seg)] = False
                Bcols[i * C : (i + 1) * C] = ids
                meta.append(ids)
            else:
                meta.append(None)
        A_core = np.zeros((KP, S * 128), dtype=ml_dtypes.bfloat16)
        A_core[:14] = A14[:, gidx]
        ALg = AL4[gidx].copy()
        ALg[alz] = 0
        AL_core = np.ascontiguousarray(
            ALg.reshape(S, 128, 4).transpose(1, 0, 2).reshape(128, S * 4)
        )
        B_core = np.zeros((KP, ncell * C), dtype=ml_dtypes.bfloat16)
        B_core[:14] = B14[:, Bcols]
        core_maps.append({"A": A_core, "B": B_core, "AL": AL_core})
        core_meta.append(meta)

    return {
        "bias": bias,
        "S": S,
        "ncell": ncell,
        "core_maps": core_maps,
        "core_meta": core_meta,
    }


def _build_program(bias, S):
    import concourse.mybir as mybir
    import concourse.tile as tile
    from concourse import bacc

    fp32 = mybir.dt.float32
    bf16 = mybir.dt.bfloat16

    ncell = S // U
    ng4 = (ncell + 3) // 4
    groups = [
        list(range(g * GROUP, min(S, (g + 1) * GROUP)))
        for g in range((S + GROUP - 1) // GROUP)
    ]

    nc = bacc.Bacc(None, target_bir_lowering=False)
    A_d = nc.declare_dram_parameter("A", [KP, S * 128], bf16, isOutput=False)
    B_d = nc.declare_dram_parameter("B", [KP, ncell * C], bf16, isOutput=False)
    AL_d = nc.declare_dram_parameter("AL", [128, S * 4], bf16, isOutput=False)
    OUT_d = nc.declare_dram_parameter("out", [36, ng4 * 512], fp32, isOutput=True)

    with tile.TileContext(nc) as tc:
        with (
            tc.tile_pool(name="singles", bufs=1) as singles,
            tc.tile_pool(name="kpool", bufs=3) as kpool,
            tc.tile_pool(name="stpool", bufs=2) as stpool,
            tc.tile_pool(name="pse", bufs=2, space="PSUM") as pse,
            tc.tile_pool(name="psacc", bufs=2, space="PSUM") as psacc,
        ):
            sb_B = singles.tile([KP, ncell * C], bf16)
            sb_A = singles.tile([KP, S * 128], bf16)
            sb_AL = singles.tile([128, S * 4], bf16)
            wtile = singles.tile([128, 128], bf16)

            nc.vector.memset(wtile, 0)

            FR = [0.0, 0.0625, 0.125, 0.25, 0.375, 0.5, 0.625, 0.75, 0.875, 1.0]
            asl = [int(f * S * 128) for f in FR]
            bsl = [int(f * ncell * C) for f in FR]
            for i in range(len(FR) - 1):
                eng = nc.sync if i % 2 == 0 else nc.gpsimd
                eng.dma_start(
                    out=sb_B[:, bsl[i] : bsl[i + 1]], in_=B_d[:, bsl[i] : bsl[i + 1]]
                )
                eng.dma_start(
                    out=sb_A[:, asl[i] : asl[i + 1]], in_=A_d[:, asl[i] : asl[i + 1]]
                )
                if i == 0:
                    nc.gpsimd.dma_start(out=sb_AL, in_=AL_d[:])

            ew = pse.tile([128, GROUP * C], fp32, name="e")
            for i in range(NWARM):
                nc.tensor.matmul(
                    ew[:, :128], lhsT=wtile, rhs=wtile, start=True, stop=True
                )

            acc = None
            for gs in groups:
                W = len(gs) * C
                e = pse.tile([128, GROUP * C], fp32, name="e")
                for i, s in enumerate(gs):
                    cell = s // U
                    nc.tensor.matmul(
                        e[:, i * C : (i + 1) * C],
                        lhsT=sb_A[:, s * 128 : (s + 1) * 128],
                        rhs=sb_B[:, cell * C : (cell + 1) * C],
                        start=True,
                        stop=True,
                    )
                k = kpool.tile([128, GROUP * C], bf16, name="k")
                nc.scalar.activation(
                    k[:, :W], e[:, :W], mybir.ActivationFunctionType.Exp,
                    bias=float(bias),
                )
                for i, s in enumerate(gs):
                    cell = s // U
                    g4, r = cell // 4, cell % 4
                    if acc is None:
                        acc = psacc.tile([36, 512], fp32, name="acc")
                    nc.tensor.matmul(
                        acc[32 * (r // 2) : 32 * (r // 2) + 4,
                            256 * (r % 2) : 256 * (r % 2) + 256],
                        lhsT=sb_AL[:, s * 4 : (s + 1) * 4],
                        rhs=k[:, i * C : (i + 1) * C],
                        start=(s % U == 0),
                        stop=(s % U == U - 1),
                    )
                    if s % U == U - 1 and (r == 3 or cell == ncell - 1):
                        st = stpool.tile([36, 512], fp32, name="st")
                        nc.vector.tensor_copy(st, acc)
                        nc.sync.dma_start(
                            out=OUT_d[:, g4 * 512 : (g4 + 1) * 512], in_=st
                        )
                        acc = None
    nc.compile()
    return nc


def _unpack(results, prep):
    out = np.zeros((M, 2), dtype=np.float32)
    for c in range(NCORES):
        o = results[c]["out"]
        for i, ids in enumerate(prep["core_meta"][c]):
            if ids is None:
                continue
            g4, r = i // 4, i % 4
            blk = o[
                32 * (r // 2) : 32 * (r // 2) + 4,
                g4 * 512 + 256 * (r % 2) : g4 * 512 + 256 * (r % 2) + C,
            ]
            out[ids, 0] += blk[0] + blk[1]
            out[ids, 1] += blk[2] + blk[3]
    return out


def kernel(X_test, X_train, alpha, log_lengthscale, log_outputscale):
    from concourse.bass_utils import run_bass_kernel_spmd

    prep = _prep(X_test, X_train, alpha, log_lengthscale, log_outputscale)

    key = (prep["S"], prep["bias"])
    if key not in _cache:
        _cache[key] = _build_program(prep["bias"], prep["S"])
    nc = _cache[key]

    core_ids = list(range(NCORES))
    res = run_bass_kernel_spmd(nc, prep["core_maps"], core_ids)
    return _unpack(res.results, prep)
